# revision 40
# baseline (speedup 1.0000x reference)
"""Bass/Trainium2 kernel for LightweightHypersphericalAttention (v4).

Sharding: 8 cores = (batch b in 0..3) x (query half in 0..1).
Each core gets x_sh [1024, 512] (its query rows), ctx [2048, 512] (full
context for its batch), the weights, and radius-derived scales; computes
its [1024, 512] slice of the final output. No collectives.

v4 structure (vs v3):
  - Host pre-casts x/ctx/W/Wp to bf16; WT/xT/cT/WpT are built with single
    DMA XBAR transposes (dma_start_transpose) straight from DRAM — no
    compute-engine casts, no PE transposes, no psum->sbuf transpose copies.
  - fq/fk/1-per-head row layout [H, 512] end to end: sqrt(ss * r^-2) on
    ACT + reciprocal on DVE, no transposes anywhere in the norm chains.
  - fk is folded into kT right after the k projection (broadcast matmul +
    DVE multiply), so the flash exp needs no per-j scale operand and can
    process TWO key chunks per ACT instruction (1024-wide exp).
  - Softmax denominator: pairwise tree adds over the 2-wide PT tiles
    (7 DVE adds per (nt,h) instead of 15 chained adds).
  - Output projection of query half 0 interleaved into half 1's flash.
"""

import numpy as np

P = 128
B, N, M, C, H = 4, 2048, 2048, 512, 4
D_V = 128
SCALE = float(D_V) ** -0.5
N_CORE = 1024          # query rows per core
MM = M // P            # 16 key chunks
CCH = C // P           # 4 channel chunks
NT = N_CORE // 512     # 2 query 512-tiles
MT = M // 512          # 4 key 512-tiles
JJ = MM // 2           # 8 key-chunk pairs per flash half
SHIFT = 1.0            # exp logit shift: keeps fp16 PT under 65504 even at
                       # the theoretical |logit| <= r^2*SCALE = 11.31 bound

_NC_CACHE = {}


def _build():
    import concourse.bass as bass
    import concourse.mybir as mybir
    import concourse.tile as tile
    from concourse import bacc
    from concourse.masks import make_identity

    f32 = mybir.dt.float32
    bf16 = mybir.dt.bfloat16
    f16 = mybir.dt.float16
    mult_op = mybir.AluOpType.mult
    add_op = mybir.AluOpType.add
    div_op = mybir.AluOpType.divide

    nc = bacc.Bacc(None, target_bir_lowering=False, debug=False)
    x_t = nc.dram_tensor("x_sh", [N_CORE, C], bf16, kind="ExternalInput")
    c_t = nc.dram_tensor("ctx", [M, C], bf16, kind="ExternalInput")
    wq_t = nc.dram_tensor("w_qkv", [2 * C, C], bf16, kind="ExternalInput")
    wp_t = nc.dram_tensor("w_proj", [C, C], bf16, kind="ExternalInput")
    sq_t = nc.dram_tensor("sq_scale", [H, 1], f32, kind="ExternalInput")
    sk_t = nc.dram_tensor("sk_scale", [H, 1], f32, kind="ExternalInput")
    out_t = nc.dram_tensor("out_sh", [N_CORE, C], f32, kind="ExternalOutput")

    from contextlib import ExitStack
    with tile.TileContext(nc) as tc, ExitStack() as es:
        const = es.enter_context(tc.tile_pool(name="const", bufs=1))
        wpool = es.enter_context(tc.tile_pool(name="wpool", bufs=1))
        big = es.enter_context(tc.tile_pool(name="big", bufs=1))
        fp = es.enter_context(tc.tile_pool(name="fp", bufs=1))
        sqp = es.enter_context(tc.tile_pool(name="sqp", bufs=2))
        fkb_p = es.enter_context(tc.tile_pool(name="fkb", bufs=2))
        ptp = es.enter_context(tc.tile_pool(name="ptp", bufs=3))
        tp = es.enter_context(tc.tile_pool(name="tp", bufs=6))
        outp = es.enter_context(tc.tile_pool(name="outp", bufs=2))
        ps_s = es.enter_context(tc.tile_pool(name="ps_s", bufs=2, space="PSUM"))
        ps_av = es.enter_context(tc.tile_pool(name="ps_av", bufs=2,
                                              space="PSUM"))
        ps_sd = es.enter_context(tc.tile_pool(name="ps_sd", bufs=1,
                                              space="PSUM"))
        ps_m = es.enter_context(tc.tile_pool(name="ps_m", bufs=1, space="PSUM"))

        # ---- front-end DMAs first: XBAR transposes + natural v load ----
        # All XBAR transposes on the sync hwdge ring (queue 0), split into
        # 512-row blocks in consumption order so each projection phase can
        # start as soon as its block lands. v rides gpsimd's software DGE
        # (a different queue) in parallel; mixing a plain DMA onto the
        # transpose ring corrupts completion tracking.
        WT = wpool.tile([P, 2, CCH, 512], bf16)    # WT[c%128, half, cc, o']
        xT = big.tile([P, NT, CCH, 512], bf16, tag="xT")
        cT = big.tile([P, MT, CCH, 512], bf16, tag="cT")
        WpT = wpool.tile([P, CCH, C], bf16)
        nc.sync.dma_start_transpose(WT[:, 0], wq_t[0:512, :])
        nc.sync.dma_start_transpose(xT[:, 0], x_t[0:512, :])
        nc.sync.dma_start_transpose(WT[:, 1], wq_t[512:1024, :])
        nc.sync.dma_start_transpose(xT[:, 1], x_t[512:1024, :])
        rqs = const.tile([H, 1], f32)
        nc.sync.dma_start(out=rqs, in_=sq_t[:])
        rks = const.tile([H, 1], f32)
        nc.sync.dma_start(out=rks, in_=sk_t[:])
        for cmt in range(MT):
            nc.sync.dma_start_transpose(cT[:, cmt],
                                        c_t[cmt * 512:(cmt + 1) * 512, :])
        nc.sync.dma_start_transpose(WpT, wp_t[:])
        # Delay v's gpsimd trigger until the transposes have drained: give
        # the v DMA a WAW dependency on a copy that itself reads the last
        # cT block (the scheduler keeps true dependencies in order).
        v_sb = big.tile([P, MM, C], bf16, tag="v")
        nc.gpsimd.tensor_copy(out=v_sb[0:1, 0, 0:2],
                              in_=cT[0:1, MT - 1, 0, 0:2])
        nc.gpsimd.dma_start(out=v_sb,
                            in_=c_t[:].rearrange("(mm p) c -> p mm c", p=P))

        # ---- constants (built while DMAs stream) ----
        # Dss[:, h, :]: ones in column h -> matmul sums partitions into row h
        Dss = const.tile([P, H, H], bf16)
        nc.vector.memset(Dss, 0.0)
        for h in range(H):
            nc.vector.memset(Dss[:, h, h:h + 1], 1.0)
        Dden = const.tile([P, H, H], f16)
        nc.vector.memset(Dden, 0.0)
        for h in range(H):
            nc.vector.memset(Dden[:, h, h:h + 1], 1.0)
        # Dbc4[:, r, :]: [4, 128] stationary broadcasting row r of a [4, n]
        # moving tensor to all 128 output partitions.
        identity = const.tile([P, P], f32)
        make_identity(nc, identity)
        Dbc4 = const.tile([H, H, P], bf16)
        for r in range(H):
            nc.vector.tensor_copy(
                out=Dbc4[:, r, :],
                in_=identity[0:H, r:r + 1].to_broadcast((H, P)))
        bias_t = const.tile([P, 1], f32)
        nc.vector.memset(bias_t, -SHIFT)

        qT = big.tile([P, 2 * H, N_CORE], bf16, tag="qT")
        kT = big.tile([P, 2 * H, M], bf16, tag="kT")
        outcatT = big.tile([P, H, N_CORE], bf16, tag="ocT")

        copy_engines = [
            lambda out, in_: nc.vector.tensor_copy(out=out, in_=in_),
            lambda out, in_: nc.scalar.copy(out=out, in_=in_),
        ]

        # ---- q projection per half nt + row norm + in-place fq scale ----
        def qproj_half(nt):
            ns = slice(nt * 512, (nt + 1) * 512)
            for dp in range(H):
                psq = ps_s.tile([P, 2, 512], f32, tag="s2",
                                name=f"psq{nt}{dp}")
                for dj in range(2):
                    do = 2 * dp + dj
                    for cc in range(CCH):
                        nc.tensor.matmul(
                            psq[:, dj, :],
                            WT[:, do // 4, cc,
                               (do % 4) * P:(do % 4 + 1) * P],
                            xT[:, nt, cc, :], start=(cc == 0),
                            stop=(cc == CCH - 1), skip_group_check=True)
                # all on DVE: ACT is busy issuing the x/ctx DMAs early on
                nc.vector.tensor_copy(out=qT[:, 2 * dp:2 * dp + 2, ns],
                                      in_=psq)
            # row sums of squares -> ss rows r=h
            ps_ss = ps_sd.tile([H, 512], f32, tag="sd", name=f"ssq{nt}")
            for h in range(H):
                sqt = sqp.tile([P, 2, 512], bf16, tag="sq")
                nc.vector.tensor_tensor(sqt, qT[:, 2 * h:2 * h + 2, ns],
                                        qT[:, 2 * h:2 * h + 2, ns], mult_op)
                for dc in range(2):
                    nc.tensor.matmul(
                        ps_ss, Dss[:, h, :], sqt[:, dc, :],
                        start=(h == 0 and dc == 0),
                        stop=(h == H - 1 and dc == 1),
                        skip_group_check=True)
            # fq = r_h / ||q|| = sqrt(r^2 * (1/ss)): fast reciprocal on DVE
            # (custom op, no act table), sqrt on ACT (sqrt table shared by
            # the whole projection phase).
            iq = fp.tile([H, 512], f32, tag="iq", name=f"iq{nt}")
            nc.vector.reciprocal_approx_fast(out=iq, in_=ps_ss)
            fq = fp.tile([H, 512], bf16, tag="fq", name=f"fq{nt}")
            nc.scalar.activation(fq, iq, mybir.ActivationFunctionType.Sqrt,
                                 scale=rqs[:, 0:1])
            for h in range(H):
                ps_b = ps_m.tile([P, 512], f32, tag="m", name=f"psbq{nt}{h}")
                nc.tensor.matmul(ps_b, Dbc4[:, h, :], fq,
                                 start=True, stop=True)
                nc.vector.tensor_tensor(
                    qT[:, 2 * h:2 * h + 2, ns],
                    qT[:, 2 * h:2 * h + 2, ns],
                    ps_b[:, None, :].to_broadcast((P, 2, 512)), mult_op)

        # ---- k projection per key tile mt + row norm + fk fold into kT ----
        def kproj_mt(mt):
            ms = slice(mt * 512, (mt + 1) * 512)
            for dp in range(H):
                psk = ps_s.tile([P, 2, 512], f32, tag="s2",
                                name=f"psk{mt}{dp}")
                for dj in range(2):
                    do = 2 * dp + dj
                    for cc in range(CCH):
                        nc.tensor.matmul(
                            psk[:, dj, :],
                            WT[:, do // 4, cc,
                               (do % 4) * P:(do % 4 + 1) * P],
                            cT[:, mt, cc, :], start=(cc == 0),
                            stop=(cc == CCH - 1), skip_group_check=True)
                copy_engines[dp % 2](kT[:, 2 * dp:2 * dp + 2, ms], psk)
            ps_ss = ps_sd.tile([H, 512], f32, tag="sd", name=f"ssk{mt}")
            for h in range(H):
                sqt = sqp.tile([P, 2, 512], bf16, tag="sq")
                nc.vector.tensor_tensor(sqt, kT[:, 2 * h:2 * h + 2, ms],
                                        kT[:, 2 * h:2 * h + 2, ms], mult_op)
                for dc in range(2):
                    nc.tensor.matmul(
                        ps_ss, Dss[:, h, :], sqt[:, dc, :],
                        start=(h == 0 and dc == 0),
                        stop=(h == H - 1 and dc == 1),
                        skip_group_check=True)
            # fk = r_h * SCALE / ||k|| = sqrt((r*SCALE)^2 * (1/ss)); kT *= fk:
            # broadcast to [P, 512] psum, bounce to sbuf bf16 on ACT, then
            # 2x-mode DVE multiply in place.
            ik = fp.tile([H, 512], f32, tag="ik", name=f"ik{mt}")
            nc.vector.reciprocal_approx_fast(out=ik, in_=ps_ss)
            fk = fp.tile([H, 512], bf16, tag="fk", name=f"fk{mt}")
            nc.scalar.activation(fk, ik, mybir.ActivationFunctionType.Sqrt,
                                 scale=rks[:, 0:1])
            for h in range(H):
                ps_b = ps_m.tile([P, 512], f32, tag="m", name=f"psbk{mt}{h}")
                nc.tensor.matmul(ps_b, Dbc4[:, h, :], fk,
                                 start=True, stop=True)
                fkb = fkb_p.tile([P, 512], bf16, tag="fkb")
                nc.scalar.copy(out=fkb, in_=ps_b)
                nc.vector.tensor_tensor(
                    kT[:, 2 * h:2 * h + 2, ms],
                    kT[:, 2 * h:2 * h + 2, ms],
                    fkb[:, None, :].to_broadcast((P, 2, 512)), mult_op)

        qproj_half(0)
        qproj_half(1)
        for mt in range(MT):
            kproj_mt(mt)

        rdens = [None, None]

        def outproj_block(nt, b):
            # outcatT is already normalized: plain psum accumulation over h
            nn = nt * 4 + b
            ps_o = ps_av.tile([P, C], f32, tag="av", name=f"pso{nt}{b}")
            for h in range(H):
                nc.tensor.matmul(ps_o, outcatT[:, h, nn * P:(nn + 1) * P],
                                 WpT[:, h, :], start=(h == 0),
                                 stop=(h == H - 1))
            acc_o = outp.tile([P, C], f32, tag="acco")
            nc.scalar.copy(out=acc_o, in_=ps_o)
            nc.scalar.dma_start(out=out_t[nn * P:(nn + 1) * P, :],
                                in_=acc_o)

        def scale_outcat(nt, tail=False):
            # outcatT[:, h, nt half] *= 1/den broadcast across partitions.
            # In the tail (flash done) the ps_s pool is idle — borrow its
            # two buffers so broadcasts pipeline with the scale TTs.
            rr = rdens[nt]
            ns = slice(nt * 512, (nt + 1) * 512)
            for h in range(H):
                if tail:
                    ps_b2 = ps_s.tile([P, 2, 512], f32, tag="s2",
                                      name=f"psbo{nt}{h}")
                    ps_b = ps_b2[:, 0, :]
                else:
                    ps_b = ps_m.tile([P, 512], f32, tag="m",
                                     name=f"psbo{nt}{h}")
                nc.tensor.matmul(ps_b, Dbc4[:, h, :], rr,
                                 start=True, stop=True,
                                 skip_group_check=True)
                nc.vector.tensor_tensor(
                    outcatT[:, h, ns], outcatT[:, h, ns], ps_b, mult_op)

        def flash_half(nt, after_head=None):
            ns = slice(nt * 512, (nt + 1) * 512)
            den_ps = ps_sd.tile([H, 512], f32, tag="sd", name=f"den{nt}")
            for h in range(H):
                avo = ps_av.tile([P, 512], f32, tag="av", name=f"avo{nt}{h}")
                t_lvl1 = []
                prev_pt = None
                for jj in range(JJ):
                    psS = ps_s.tile([P, 2, 512], f32, tag="s2",
                                    name=f"psS{nt}{h}{jj}")
                    for dj in range(2):
                        j = 2 * jj + dj
                        nc.tensor.matmul(
                            psS[:, dj, :], kT[:, 2 * h, j * P:(j + 1) * P],
                            qT[:, 2 * h, ns], start=True, stop=False,
                            skip_group_check=True)
                        nc.tensor.matmul(
                            psS[:, dj, :],
                            kT[:, 2 * h + 1, j * P:(j + 1) * P],
                            qT[:, 2 * h + 1, ns], start=False, stop=True,
                            skip_group_check=True)
                    PT = ptp.tile([P, 2, 512], f16, tag="pt")
                    nc.scalar.activation(
                        PT, psS, mybir.ActivationFunctionType.Exp,
                        bias=bias_t[:, 0:1])
                    nc.tensor.matmul(avo, v_sb[:, 2 * jj, h * P:(h + 1) * P],
                                     PT[:, 0, :], start=(jj == 0),
                                     stop=False)
                    nc.tensor.matmul(avo,
                                     v_sb[:, 2 * jj + 1,
                                          h * P:(h + 1) * P],
                                     PT[:, 1, :], start=False,
                                     stop=(jj == JJ - 1))
                    # pairwise denominator tree on DVE (f16 sbuf 2x mode)
                    if jj % 2 == 0:
                        prev_pt = PT
                    else:
                        t = tp.tile([P, 2, 512], f16, tag="t")
                        nc.vector.tensor_tensor(t, prev_pt, PT, add_op)
                        t_lvl1.append(t)
                u0 = tp.tile([P, 2, 512], f16, tag="t")
                nc.vector.tensor_tensor(u0, t_lvl1[0], t_lvl1[1], add_op)
                u1 = tp.tile([P, 2, 512], f16, tag="t")
                nc.vector.tensor_tensor(u1, t_lvl1[2], t_lvl1[3], add_op)
                acc = tp.tile([P, 2, 512], f16, tag="t")
                nc.vector.tensor_tensor(acc, u0, u1, add_op)
                for dc in range(2):
                    nc.tensor.matmul(den_ps, Dden[:, h, :], acc[:, dc, :],
                                     start=(h == 0 and dc == 0),
                                     stop=(h == H - 1 and dc == 1),
                                     skip_group_check=True)
                nc.vector.tensor_copy(out=outcatT[:, h, ns], in_=avo)
                if after_head is not None:
                    after_head(h)
            # 1/den: fast reciprocal on DVE, bf16 bounce via ACT copy
            # ('copy' lives in every act table — no swap mid-flash).
            rd32 = fp.tile([H, 512], f32, tag="rd32", name=f"rd32{nt}")
            nc.vector.reciprocal_approx_fast(out=rd32, in_=den_ps)
            rden = fp.tile([H, 512], bf16, tag="rden", name=f"rden{nt}")
            nc.scalar.copy(out=rden, in_=rd32)
            rdens[nt] = rden

        def after_head_nt1(h):
            if h == 0:
                scale_outcat(0)
            outproj_block(0, h)

        flash_half(0)
        # interleave half 0's normalize+projection into half 1's flash
        flash_half(1, after_head=after_head_nt1)
        scale_outcat(1, tail=True)
        for b in range(4):
            outproj_block(1, b)

    nc.compile()
    return nc


def _get_nc():
    if "v4" not in _NC_CACHE:
        _NC_CACHE["v4"] = _build()
    return _NC_CACHE["v4"]


def kernel(x, context, W_qkv, W_proj, radius, _trace=False, _bf16=True):
    import ml_dtypes
    from concourse.bass_utils import run_bass_kernel_spmd

    bf = ml_dtypes.bfloat16
    x = np.asarray(x, dtype=np.float32)
    context = np.asarray(context, dtype=np.float32)
    W_qkv_b = np.ascontiguousarray(np.asarray(W_qkv, dtype=np.float32)
                                   .astype(bf))
    W_proj_b = np.ascontiguousarray(np.asarray(W_proj, dtype=np.float32)
                                    .astype(bf))
    radius = np.asarray(radius, dtype=np.float32)
    sq_scale = np.ascontiguousarray(
        (radius ** 2).reshape(H, 1).astype(np.float32))
    sk_scale = np.ascontiguousarray(
        ((radius * SCALE) ** 2).reshape(H, 1).astype(np.float32))

    x_b = np.ascontiguousarray(x.astype(bf))
    ctx_b = np.ascontiguousarray(context.astype(bf))

    nc = _get_nc()
    in_maps = []
    for i in range(8):
        b, half = i // 2, i % 2
        in_maps.append({
            "x_sh": x_b[b, half * N_CORE:(half + 1) * N_CORE, :],
            "ctx": ctx_b[b],
            "w_qkv": W_qkv_b,
            "w_proj": W_proj_b,
            "sq_scale": sq_scale,
            "sk_scale": sk_scale,
        })
    res = run_bass_kernel_spmd(nc, in_maps, list(range(8)), trace=_trace)
    out = np.empty((B, N, C), dtype=np.float32)
    for i in range(8):
        b, half = i // 2, i % 2
        out[b, half * N_CORE:(half + 1) * N_CORE, :] = res.results[i]["out_sh"]
    if _trace:
        return out, res
    return out
